# revision 26
# baseline (speedup 1.0000x reference)
"""Trainium2 Bass kernel for the two-stream cosine (linear) attention module.

Reference computation (per batch b):
    qx,kx,vx = l2norm_h(x @ Wq_x^T), l2norm_h(x @ Wk_x^T), x @ Wv_x^T
    qc,kc,vc = same with c and the *_c weights
    q,k,v    = concat over sequence;  v /= n_tot**sigmoid(norm_const)
    kv   = sum_n k[n]^T v[n]  (per head, 64x64)
    attn = q @ kv
    out_x = attn_x @ Wo_x^T ; out_c = attn_c @ Wo_c^T

Sharding: 8 cores = (4 batches) x (2 sequence halves). Each core projects its
2048 x-rows + 128 c-rows, accumulates a partial per-head kv in PSUM, does a
pairwise (per-batch) AllReduce of the diagonal kv blocks (256 KiB),
then computes
    out = q_hat @ (blockdiag(kv) @ Wo^T)
so the (n_tot x 1024) attn intermediate is never materialized.

Compute: bf16 TensorEngine matmuls with fp32 PSUM accumulation; the L2 norms
are computed in fp32 from PSUM. Host pre-transposes x/c/W (free) and folds the
n_tot**sigmoid(norm_const) scale into Wv.
"""

import numpy as np

B = 4
N = 4096
MC = 256
D = 1024
H = 16
HD = 64
NCORES = 8
NTOT = N + MC                 # 4352
DT = D // 128                 # 8 din tiles

_CACHE: dict = {}


def _build_nc(TX=N * B // NCORES, TC=MC * B // NCORES, use_cc=True, dbg=False):
    """Build the SPMD per-core graph. TX/TC: per-core x/c rows (mult of 128)."""
    import concourse.tile as tile
    from concourse import bacc, mybir
    from concourse.masks import make_identity

    BF16 = mybir.dt.bfloat16
    F32 = mybir.dt.float32
    AX = mybir.AxisListType
    XTILES = TX // 128
    TTILES = XTILES + 1

    nc = bacc.Bacc("TRN2", target_bir_lowering=False, debug=False,
                   enable_asserts=False, num_devices=NCORES)

    xT_d = nc.dram_tensor("xT", [XTILES, 128, DT, 128], BF16, kind="ExternalInput")
    cT_d = nc.dram_tensor("cT", [128, DT, TC], BF16, kind="ExternalInput")
    WNAMES = ["wqxT", "wkxT", "wvxT", "wqcT", "wkcT", "wvcT", "woxT", "wocT"]
    wd = {n: nc.dram_tensor(n, [D, D], BF16, kind="ExternalInput") for n in WNAMES}
    outx_d = nc.dram_tensor("out_x", [TX, D], F32, kind="ExternalOutput")
    outc_d = nc.dram_tensor("out_c", [TC, D], F32, kind="ExternalOutput")
    if dbg:
        dbg_khat = nc.dram_tensor("dbg_khat", [128, D], BF16, kind="ExternalOutput")
        dbg_vsb = nc.dram_tensor("dbg_vsb", [128, D], BF16, kind="ExternalOutput")
        dbg_kv = nc.dram_tensor("dbg_kv", [128, 8, 128], F32, kind="ExternalOutput")  # cols 64: unused
        dbg_m = nc.dram_tensor("dbg_m", [128, DT, D], BF16, kind="ExternalOutput")
        dbg_qt = nc.dram_tensor("dbg_qt", [128, DT, TX + TC], BF16, kind="ExternalOutput")
        dbg_kvpre = nc.dram_tensor("dbg_kvpre", [128, 8, 128], F32, kind="ExternalOutput")

    with tile.TileContext(nc) as tc:
        with (
            tc.tile_pool(name="pbig", bufs=1) as pbig,
            tc.tile_pool(name="pw", bufs=4) as pw,
            tc.tile_pool(name="pact", bufs=3) as pact,
            tc.tile_pool(name="pkh", bufs=6) as pkh,
            tc.tile_pool(name="psmall", bufs=1) as psmall,
            tc.tile_pool(name="psA", bufs=4, space="PSUM") as psA,
            tc.tile_pool(name="psKV", bufs=1, space="PSUM") as psKV,
            tc.tile_pool(name="psT", bufs=2, space="PSUM") as psT,
            tc.tile_pool(name="pdram", bufs=1, space="DRAM") as pdram,
        ):
            # ---------- resident loads ----------
            def load_w(name):
                t = pw.tile([128, DT, D], BF16, tag="w")
                for d in range(DT):
                    nc.sync.dma_start(out=t[:, d, :], in_=wd[name][d * 128:(d + 1) * 128, :])
                return t

            # DMA order tuned for fast PE start: x tile 0 first, then wk/wv
            # in half-column chunks (tile 0's matmuls unlock per-slice), then
            # the remaining x tiles stream in ahead of compute.
            EARLY = min(4, XTILES)
            xsb = pbig.tile([128, DT, TX], BF16, tag="xsb")       # 32 KiB/p
            nc.sync.dma_start(out=xsb[:, :, 0:128], in_=xT_d[0])
            wk = pw.tile([128, DT, D], BF16, tag="w")
            wv = pw.tile([128, DT, D], BF16, tag="w")
            for d in range(DT):
                nc.sync.dma_start(out=wk[:, d, :],
                                  in_=wd["wkxT"][d * 128:(d + 1) * 128, :])
            for t in range(1, EARLY):
                nc.sync.dma_start(out=xsb[:, :, t * 128:(t + 1) * 128], in_=xT_d[t])
            for d in range(DT):
                nc.sync.dma_start(out=wv[:, d, :],
                                  in_=wd["wvxT"][d * 128:(d + 1) * 128, :])
            for t in range(EARLY, XTILES):
                nc.sync.dma_start(out=xsb[:, :, t * 128:(t + 1) * 128], in_=xT_d[t])
            csb = pbig.tile([128, DT, TC], BF16, tag="csb")
            nc.sync.dma_start(out=csb[:, :, :], in_=cT_d[:, :, :])

            ident = pbig.tile([128, 128], BF16, tag="ident")
            make_identity(nc, ident)
            eps = pbig.tile([128, 1], F32, tag="eps")
            nc.vector.memset(eps[:], 1e-24)

            # PE warmup: the HAM clock gate starts at 1.2 GHz and needs ~3.4us
            # of sustained activity to release. Burn junk matmuls during the
            # engine-boot dead window (before the first weight DMAs land) so
            # the real matmuls start at 2.4 GHz.
            junk = pbig.tile([128, 512], BF16, tag="junk")
            nc.vector.memset(junk[:], 0.0)
            for w_ in range(10):
                jp = psT.tile([128, 512], F32, tag="tps")
                nc.tensor.matmul(jp[:], junk[:, 0:128], junk[:],
                                 start=True, stop=True, skip_group_check=True)

            # persistent kv accumulator: 8 pair-blocks [128v x 128k]
            kv_ps = psKV.tile([128, 8, 128], F32, tag="kv")

            def src_of(t):
                if t == XTILES:
                    return csb, 0
                return xsb, t * 128

            def l2norm_scaled(proj_ps, dst, half, tag_prefix=""):
                """dst[:, half*512 + h*64 : ...] = normalize per 64-dim head."""
                # walrus: only ONE non-scalar PSUM input per DVE op, so square
                # on the scalar engine (single input) instead of tensor_mul.
                sq = pact.tile([128, 512], F32, tag="sq")
                nc.scalar.activation(sq[:], proj_ps[:],
                                     mybir.ActivationFunctionType.Square)
                ss = pact.tile([128, 8], F32, tag="ss")
                nc.vector.reduce_sum(
                    out=ss[:], in_=sq[:].rearrange("p (h e) -> p h e", e=HD),
                    axis=AX.X)
                sn = pact.tile([128, 8], F32, tag="sn")
                nc.scalar.activation(sn[:], ss[:],
                                     mybir.ActivationFunctionType.Sqrt, bias=eps[:])
                rstd = pact.tile([128, 8], F32, tag="rstd")
                nc.vector.reciprocal(rstd[:], sn[:])
                # one broadcast multiply for all 8 heads: rstd [128,8] read
                # with a stride-0 inner dim against the [128,8,64] projection
                nc.vector.tensor_mul(
                    dst[:, half * 512:(half + 1) * 512].rearrange(
                        "p (h e) -> p h e", e=HD),
                    proj_ps[:].rearrange("p (h e) -> p h e", e=HD),
                    rstd[:].to_broadcast((128, 8, HD)))

            # ---------- phase KV: k,v projections + kv accumulation ----------
            # The first EARLY tiles' k-projections run while wv is still in
            # flight (they only need wk), flattening the startup DMA ramp.
            early_khat = {}
            for t in range(EARLY):
                src, col0 = src_of(t)
                khat = pkh.tile([128, D], BF16, tag="khat")
                for half in range(2):
                    hs = slice(half * 512, (half + 1) * 512)
                    kp = psA.tile([128, 512], F32, tag="proj")
                    for d in range(DT):
                        nc.tensor.matmul(kp[:], src[:, d, col0:col0 + 128],
                                         wk[:, d, hs], start=(d == 0), stop=(d == DT - 1))
                    l2norm_scaled(kp, khat, half)
                    # keep the PE duty cycle above the HAM re-throttle
                    # threshold while the ramp is DMA-paced (junk fills gaps)
                    for w_ in range(3):
                        jp = psT.tile([128, 512], F32, tag="tps")
                        nc.tensor.matmul(jp[:], junk[:, 0:128], junk[:],
                                         start=True, stop=True,
                                         skip_group_check=True)
                early_khat[t] = khat
            for t in range(TTILES):
                if t == XTILES:
                    wk = load_w("wkcT")
                    wv = load_w("wvcT")
                src, col0 = src_of(t)
                khat = early_khat.get(t)
                vsb = pact.tile([128, D], BF16, tag="vsb")
                for half in range(2):
                    hs = slice(half * 512, (half + 1) * 512)
                    if t >= EARLY:
                        if half == 0:
                            khat = pkh.tile([128, D], BF16, tag="khat")
                        kp = psA.tile([128, 512], F32, tag="proj")
                        for d in range(DT):
                            nc.tensor.matmul(kp[:], src[:, d, col0:col0 + 128],
                                             wk[:, d, hs], start=(d == 0), stop=(d == DT - 1))
                        l2norm_scaled(kp, khat, half)
                    vp = psA.tile([128, 512], F32, tag="proj")
                    for d in range(DT):
                        nc.tensor.matmul(vp[:], src[:, d, col0:col0 + 128],
                                         wv[:, d, hs], start=(d == 0), stop=(d == DT - 1))
                    # v copy-cast to bf16 (n_tot scale folded into Wv on host)
                    nc.scalar.activation(vsb[:, hs], vp[:],
                                         mybir.ActivationFunctionType.Copy)
                if dbg and t == 0:
                    nc.sync.dma_start(out=dbg_khat[:, :], in_=khat[:])
                    nc.sync.dma_start(out=dbg_vsb[:, :], in_=vsb[:])
                for half in range(2):
                    hs = slice(half * 512, (half + 1) * 512)
                    # kv accumulation over tiles: out[vdim, kdim] = v^T k_hat
                    # start=True clears has_written for the WHOLE bank, so only
                    # the first matmul of each bank (g==0; halves are separate
                    # banks) may set it. Later blocks overwrite (bit clear) on
                    # t==0 and accumulate (bit set) afterwards.
                    for g in range(4):
                        gg = half * 4 + g
                        ps_ = slice(half * 512 + g * 128, half * 512 + (g + 1) * 128)
                        nc.tensor.matmul(kv_ps[:, gg, :], vsb[:, ps_], khat[:, ps_],
                                         start=(t == 0 and g == 0),
                                         stop=(t == TTILES - 1),
                                         skip_group_check=True)

            # kv -> DRAM bounce -> pairwise AllReduce (diagonal blocks only:
            # pair block rows 0:64 <-> cols 0:64 (head 2g), 64:128 <-> 64:128)
            kv_stage = psmall.tile([128, 8, 64], F32, tag="kvstage")
            nc.vector.tensor_copy(kv_stage[0:64, :, :], kv_ps[0:64, :, 0:64])
            nc.vector.tensor_copy(kv_stage[64:128, :, :], kv_ps[64:128, :, 64:128])
            bounce_in = pdram.tile([128, 8 * 64], F32)
            bounce_out = pdram.tile([128, 8 * 64], F32)
            wq = load_w("wqxT")
            nc.gpsimd.dma_start(out=bounce_in[:], in_=kv_stage[:])
            if dbg:
                nc.sync.dma_start(out=dbg_kvpre[:, :, :], in_=kv_stage[:])
            if use_cc:
                nc.gpsimd.collective_compute(
                    "AllReduce", mybir.AluOpType.add,
                    replica_groups=[[0, 1], [2, 3], [4, 5], [6, 7]],
                    ins=[bounce_in[:].opt()],
                    outs=[bounce_out[:].opt()],
                )
            else:
                bounce_out = bounce_in

            # ---------- phase Q (overlaps the AllReduce) ----------
            qT = pbig.tile([128, DT, TX + TC], BF16, tag="qT")    # 34 KiB/p
            for t in range(TTILES):
                if t == XTILES:
                    wq = load_w("wqcT")
                src, col0 = src_of(t)
                qhat = pact.tile([128, D], BF16, tag="qhat")
                for half in range(2):
                    hs = slice(half * 512, (half + 1) * 512)
                    qp = psA.tile([128, 512], F32, tag="proj")
                    for d in range(DT):
                        nc.tensor.matmul(qp[:], src[:, d, col0:col0 + 128],
                                         wq[:, d, hs], start=(d == 0), stop=(d == DT - 1))
                    l2norm_scaled(qp, qhat, half)
                # transpose q_hat into qT storage
                tcol = t * 128 if t < XTILES else TX
                tps = psT.tile([128, DT, 128], BF16, tag="tps")
                for d in range(DT):
                    nc.tensor.matmul(tps[:, d, :], qhat[:, d * 128:(d + 1) * 128],
                                     ident[:], is_transpose=True)
                nc.scalar.activation(qT[:, :, tcol:tcol + 128], tps[:],
                                     mybir.ActivationFunctionType.Copy)

            # ---------- AllReduce result back + M build ----------
            wox = load_w("woxT")
            woc = load_w("wocT")
            kvr = psmall.tile([128, 8, 64], F32, tag="kvr")
            nc.gpsimd.dma_start(out=kvr[:], in_=bounce_out[:])
            # block-diagonal kv tile: bd[:, g, :] = [[vk_h0, 0], [0, vk_h1]]
            # so one 128-deep matmul per (pair, chunk) builds both heads' M rows
            kvb = psmall.tile([128, 8, 128], BF16, tag="kvb")
            nc.vector.memset(kvb[:], 0.0)
            nc.vector.tensor_copy(kvb[0:64, :, 0:64], kvr[0:64, :, :])
            nc.vector.tensor_copy(kvb[64:128, :, 64:128], kvr[64:128, :, :])
            if dbg:
                nc.sync.dma_start(out=dbg_kv[:, :, 0:64], in_=kvr[:])

            Mx = pbig.tile([128, DT, D], BF16, tag="xsb")   # reuse xsb slot
            Mc = pbig.tile([128, DT, D], BF16, tag="Mc")
            for wo, Mt in ((wox, Mx), (woc, Mc)):
                for g in range(8):
                    mp0 = psA.tile([128, 512], F32, tag="proj")
                    mp1 = psA.tile([128, 512], F32, tag="proj")
                    for mp, chunk in ((mp0, 0), (mp1, 1)):
                        nc.tensor.matmul(
                            mp[:], kvb[:, g, :],
                            wo[:, g, chunk * 512:(chunk + 1) * 512],
                            start=True, stop=True, skip_group_check=True)
                    nc.vector.tensor_copy(Mt[:, g, 0:512], mp0[:])
                    nc.scalar.activation(Mt[:, g, 512:1024], mp1[:],
                                         mybir.ActivationFunctionType.Copy)

            if dbg:
                nc.sync.dma_start(out=dbg_m[:, :, :], in_=Mx[:])
                nc.sync.dma_start(out=dbg_qt[:, :, :], in_=qT[:])

            # ---------- out pass ----------
            # per-half copies alternate DVE/ACT (run concurrently) and each
            # half DMAs out as soon as its copy lands, shortening the tail
            for t in range(TTILES):
                is_c = (t == XTILES)
                tcol = TX if is_c else t * 128
                Mt = Mc if is_c else Mx
                ob = pact.tile([128, D], F32, tag="ob")
                for half in range(2):
                    hs = slice(half * 512, (half + 1) * 512)
                    op = psA.tile([128, 512], F32, tag="proj")
                    for d in range(DT):
                        nc.tensor.matmul(op[:], qT[:, d, tcol:tcol + 128],
                                         Mt[:, d, hs], start=(d == 0), stop=(d == DT - 1))
                    if half == 0:
                        nc.vector.tensor_copy(ob[:, hs], op[:])
                    else:
                        nc.scalar.activation(ob[:, hs], op[:],
                                             mybir.ActivationFunctionType.Copy)
                    dst = outc_d if is_c else outx_d[t * 128:(t + 1) * 128, :]
                    nc.sync.dma_start(out=dst[:, hs], in_=ob[:, hs])

    nc.compile()
    return nc


def _prep_in_maps(x, c, Wq_x, Wk_x, Wv_x, Wq_c, Wk_c, Wv_c, Wo_x, Wo_c,
                  norm_const, TX=None, TC=None):
    import ml_dtypes
    bf = ml_dtypes.bfloat16
    n = x.shape[1]
    m = c.shape[1]
    TX = TX or n * B // NCORES
    TC = TC or m * B // NCORES
    x = np.asarray(x, np.float32)
    c = np.asarray(c, np.float32)
    norm_const = np.asarray(norm_const, np.float32)

    ntot = n + m
    sig = 1.0 / (1.0 + np.exp(-norm_const.astype(np.float64).reshape(H)))
    scale = (float(ntot) ** (-sig)).astype(np.float32)
    svec = np.repeat(scale, HD)            # (1024,) per output dim
    Wv_x_s = np.asarray(Wv_x, np.float32) * svec[:, None]
    Wv_c_s = np.asarray(Wv_c, np.float32) * svec[:, None]

    def wT(W):
        return np.ascontiguousarray(np.asarray(W, np.float32).T.astype(bf))

    weights = {
        "wqxT": wT(Wq_x), "wkxT": wT(Wk_x), "wvxT": wT(Wv_x_s),
        "wqcT": wT(Wq_c), "wkcT": wT(Wk_c), "wvcT": wT(Wv_c_s),
        "woxT": wT(Wo_x), "wocT": wT(Wo_c),
    }

    xr = x.reshape(B, 2, TX, D)
    cr = c.reshape(B, 2, TC, D)
    xtiles = TX // 128
    in_maps = []
    for i in range(NCORES):
        b, hf = i // 2, i % 2
        mm = dict(weights)
        # tile-major: [t, p(dim%128), d(dim//128), j(tok%128)] so each
        # per-tile DMA reads 2 KiB contiguous per partition row
        xs = xr[b, hf].astype(bf)
        mm["xT"] = np.ascontiguousarray(
            xs.reshape(xtiles, 128, DT, 128).transpose(0, 3, 2, 1))
        cs = cr[b, hf].astype(bf)
        mm["cT"] = np.ascontiguousarray(
            cs.reshape(TC, DT, 128).transpose(2, 1, 0))
        in_maps.append(mm)
    return in_maps


def _gather_outputs(results, n=N, m=MC):
    TX = n * B // NCORES
    TC = m * B // NCORES
    out_x = np.empty((B, 2, TX, D), np.float32)
    out_c = np.empty((B, 2, TC, D), np.float32)
    for i in range(NCORES):
        b, hf = i // 2, i % 2
        out_x[b, hf] = results[i]["out_x"]
        out_c[b, hf] = results[i]["out_c"]
    return out_x.reshape(B, n, D), out_c.reshape(B, m, D)


def kernel(x, c, Wq_x, Wk_x, Wv_x, Wq_c, Wk_c, Wv_c, Wo_x, Wo_c, norm_const):
    import os
    from concourse import bass_utils

    if "nc" not in _CACHE:
        _CACHE["nc"] = _build_nc()
    nc = _CACHE["nc"]
    in_maps = _prep_in_maps(x, c, Wq_x, Wk_x, Wv_x, Wq_c, Wk_c, Wv_c,
                            Wo_x, Wo_c, norm_const)
    kwargs = {}
    if os.environ.get("KERNEL_TRACE") == "1":
        try:
            from antenv.axon_hooks import get_axon_ntff_profile_hook
            if get_axon_ntff_profile_hook() is not None:
                kwargs = {"trace": True,
                          "tmpdir": os.environ.get("KERNEL_TRACE_DIR") or None}
        except ImportError:
            pass
    res = bass_utils.run_bass_kernel_spmd(nc, in_maps, core_ids=list(range(NCORES)),
                                          **kwargs)
    _CACHE["last_res"] = res
    return _gather_outputs(res.results)


# revision 27
# speedup vs baseline: 1.0147x; 1.0147x over previous
"""Trainium2 Bass kernel for the two-stream cosine (linear) attention module.

Reference computation (per batch b):
    qx,kx,vx = l2norm_h(x @ Wq_x^T), l2norm_h(x @ Wk_x^T), x @ Wv_x^T
    qc,kc,vc = same with c and the *_c weights
    q,k,v    = concat over sequence;  v /= n_tot**sigmoid(norm_const)
    kv   = sum_n k[n]^T v[n]  (per head, 64x64)
    attn = q @ kv
    out_x = attn_x @ Wo_x^T ; out_c = attn_c @ Wo_c^T

Sharding: 8 cores = (4 batches) x (2 sequence halves). Each core projects its
2048 x-rows + 128 c-rows, accumulates a partial per-head kv in PSUM, does a
pairwise (per-batch) AllReduce of the diagonal kv blocks (256 KiB),
then computes
    out = q_hat @ (blockdiag(kv) @ Wo^T)
so the (n_tot x 1024) attn intermediate is never materialized.

Compute: bf16 TensorEngine matmuls with fp32 PSUM accumulation; the L2 norms
are computed in fp32 from PSUM. Host pre-transposes x/c/W (free) and folds the
n_tot**sigmoid(norm_const) scale into Wv.
"""

import numpy as np

B = 4
N = 4096
MC = 256
D = 1024
H = 16
HD = 64
NCORES = 8
NTOT = N + MC                 # 4352
DT = D // 128                 # 8 din tiles

_CACHE: dict = {}


def _build_nc(TX=N * B // NCORES, TC=MC * B // NCORES, use_cc=True, dbg=False):
    """Build the SPMD per-core graph. TX/TC: per-core x/c rows (mult of 128)."""
    import concourse.tile as tile
    from concourse import bacc, mybir
    from concourse.masks import make_identity

    BF16 = mybir.dt.bfloat16
    F32 = mybir.dt.float32
    AX = mybir.AxisListType
    XTILES = TX // 128
    TTILES = XTILES + 1

    nc = bacc.Bacc("TRN2", target_bir_lowering=False, debug=False,
                   enable_asserts=False, num_devices=NCORES)

    xT_d = nc.dram_tensor("xT", [XTILES, 128, DT, 128], BF16, kind="ExternalInput")
    cT_d = nc.dram_tensor("cT", [128, DT, TC], BF16, kind="ExternalInput")
    WNAMES = ["wqxT", "wkxT", "wvxT", "wqcT", "wkcT", "wvcT", "woxT", "wocT"]
    wd = {n: nc.dram_tensor(n, [D, D], BF16, kind="ExternalInput") for n in WNAMES}
    outx_d = nc.dram_tensor("out_x", [TX, D], F32, kind="ExternalOutput")
    outc_d = nc.dram_tensor("out_c", [TC, D], F32, kind="ExternalOutput")
    if dbg:
        dbg_khat = nc.dram_tensor("dbg_khat", [128, D], BF16, kind="ExternalOutput")
        dbg_vsb = nc.dram_tensor("dbg_vsb", [128, D], BF16, kind="ExternalOutput")
        dbg_kv = nc.dram_tensor("dbg_kv", [128, 8, 128], F32, kind="ExternalOutput")  # cols 64: unused
        dbg_m = nc.dram_tensor("dbg_m", [128, DT, D], BF16, kind="ExternalOutput")
        dbg_qt = nc.dram_tensor("dbg_qt", [128, DT, TX + TC], BF16, kind="ExternalOutput")
        dbg_kvpre = nc.dram_tensor("dbg_kvpre", [128, 8, 128], F32, kind="ExternalOutput")

    with tile.TileContext(nc) as tc:
        with (
            tc.tile_pool(name="pbig", bufs=1) as pbig,
            tc.tile_pool(name="pw", bufs=4) as pw,
            tc.tile_pool(name="pact", bufs=3) as pact,
            tc.tile_pool(name="pkh", bufs=6) as pkh,
            tc.tile_pool(name="psmall", bufs=1) as psmall,
            tc.tile_pool(name="psA", bufs=4, space="PSUM") as psA,
            tc.tile_pool(name="psKV", bufs=1, space="PSUM") as psKV,
            tc.tile_pool(name="psT", bufs=2, space="PSUM") as psT,
            tc.tile_pool(name="pdram", bufs=1, space="DRAM") as pdram,
        ):
            # ---------- resident loads ----------
            def load_w(name):
                t = pw.tile([128, DT, D], BF16, tag="w")
                for d in range(DT):
                    nc.sync.dma_start(out=t[:, d, :], in_=wd[name][d * 128:(d + 1) * 128, :])
                return t

            # DMA order tuned for fast PE start: x tile 0 first, then wk/wv
            # in half-column chunks (tile 0's matmuls unlock per-slice), then
            # the remaining x tiles stream in ahead of compute.
            EARLY = min(4, XTILES)
            xsb = pbig.tile([128, DT, TX], BF16, tag="xsb")       # 32 KiB/p
            nc.sync.dma_start(out=xsb[:, :, 0:128], in_=xT_d[0])
            wk = pw.tile([128, DT, D], BF16, tag="w")
            wv = pw.tile([128, DT, D], BF16, tag="w")
            for d in range(DT):
                nc.sync.dma_start(out=wk[:, d, :],
                                  in_=wd["wkxT"][d * 128:(d + 1) * 128, :])
            for t in range(1, EARLY):
                nc.sync.dma_start(out=xsb[:, :, t * 128:(t + 1) * 128], in_=xT_d[t])
            for d in range(DT):
                nc.sync.dma_start(out=wv[:, d, :],
                                  in_=wd["wvxT"][d * 128:(d + 1) * 128, :])
            for t in range(EARLY, XTILES):
                nc.sync.dma_start(out=xsb[:, :, t * 128:(t + 1) * 128], in_=xT_d[t])
            csb = pbig.tile([128, DT, TC], BF16, tag="csb")
            nc.sync.dma_start(out=csb[:, :, :], in_=cT_d[:, :, :])

            ident = pbig.tile([128, 128], BF16, tag="ident")
            make_identity(nc, ident)
            eps = pbig.tile([128, 1], F32, tag="eps")
            nc.vector.memset(eps[:], 1e-24)

            # PE warmup: the HAM clock gate starts at 1.2 GHz and needs ~3.4us
            # of sustained activity to release. Burn junk matmuls during the
            # engine-boot dead window (before the first weight DMAs land) so
            # the real matmuls start at 2.4 GHz.
            junk = pbig.tile([128, 512], BF16, tag="junk")
            nc.vector.memset(junk[:], 0.0)
            for w_ in range(10):
                jp = psT.tile([128, 512], F32, tag="tps")
                nc.tensor.matmul(jp[:], junk[:, 0:128], junk[:],
                                 start=True, stop=True, skip_group_check=True)

            # persistent kv accumulator: 8 pair-blocks [128v x 128k]
            kv_ps = psKV.tile([128, 8, 128], F32, tag="kv")

            def src_of(t):
                if t == XTILES:
                    return csb, 0
                return xsb, t * 128

            def l2norm_scaled(proj_ps, dst, half, tag_prefix=""):
                """dst[:, half*512 + h*64 : ...] = normalize per 64-dim head."""
                # walrus: only ONE non-scalar PSUM input per DVE op, so square
                # on the scalar engine (single input) instead of tensor_mul.
                sq = pact.tile([128, 512], F32, tag="sq")
                nc.scalar.activation(sq[:], proj_ps[:],
                                     mybir.ActivationFunctionType.Square)
                ss = pact.tile([128, 8], F32, tag="ss")
                nc.vector.reduce_sum(
                    out=ss[:], in_=sq[:].rearrange("p (h e) -> p h e", e=HD),
                    axis=AX.X)
                sn = pact.tile([128, 8], F32, tag="sn")
                nc.scalar.activation(sn[:], ss[:],
                                     mybir.ActivationFunctionType.Sqrt, bias=eps[:])
                rstd = pact.tile([128, 8], F32, tag="rstd")
                nc.vector.reciprocal(rstd[:], sn[:])
                # one broadcast multiply for all 8 heads: rstd [128,8] read
                # with a stride-0 inner dim against the [128,8,64] projection
                nc.vector.tensor_mul(
                    dst[:, half * 512:(half + 1) * 512].rearrange(
                        "p (h e) -> p h e", e=HD),
                    proj_ps[:].rearrange("p (h e) -> p h e", e=HD),
                    rstd[:].to_broadcast((128, 8, HD)))

            # ---------- phase KV: k,v projections + kv accumulation ----------
            # The first EARLY tiles' k-projections run while wv is still in
            # flight (they only need wk), flattening the startup DMA ramp.
            early_khat = {}
            for t in range(EARLY):
                src, col0 = src_of(t)
                khat = pkh.tile([128, D], BF16, tag="khat")
                for half in range(2):
                    hs = slice(half * 512, (half + 1) * 512)
                    kp = psA.tile([128, 512], F32, tag="proj")
                    for d in range(DT):
                        nc.tensor.matmul(kp[:], src[:, d, col0:col0 + 128],
                                         wk[:, d, hs], start=(d == 0), stop=(d == DT - 1))
                    l2norm_scaled(kp, khat, half)
                early_khat[t] = khat
            for t in range(TTILES):
                if t == XTILES:
                    wk = load_w("wkcT")
                    wv = load_w("wvcT")
                src, col0 = src_of(t)
                khat = early_khat.get(t)
                vsb = pact.tile([128, D], BF16, tag="vsb")
                for half in range(2):
                    hs = slice(half * 512, (half + 1) * 512)
                    if t >= EARLY:
                        if half == 0:
                            khat = pkh.tile([128, D], BF16, tag="khat")
                        kp = psA.tile([128, 512], F32, tag="proj")
                        for d in range(DT):
                            nc.tensor.matmul(kp[:], src[:, d, col0:col0 + 128],
                                             wk[:, d, hs], start=(d == 0), stop=(d == DT - 1))
                        l2norm_scaled(kp, khat, half)
                    vp = psA.tile([128, 512], F32, tag="proj")
                    for d in range(DT):
                        nc.tensor.matmul(vp[:], src[:, d, col0:col0 + 128],
                                         wv[:, d, hs], start=(d == 0), stop=(d == DT - 1))
                    # v copy-cast to bf16 (n_tot scale folded into Wv on host)
                    nc.scalar.activation(vsb[:, hs], vp[:],
                                         mybir.ActivationFunctionType.Copy)
                if dbg and t == 0:
                    nc.sync.dma_start(out=dbg_khat[:, :], in_=khat[:])
                    nc.sync.dma_start(out=dbg_vsb[:, :], in_=vsb[:])
                for half in range(2):
                    hs = slice(half * 512, (half + 1) * 512)
                    # kv accumulation over tiles: out[vdim, kdim] = v^T k_hat
                    # start=True clears has_written for the WHOLE bank, so only
                    # the first matmul of each bank (g==0; halves are separate
                    # banks) may set it. Later blocks overwrite (bit clear) on
                    # t==0 and accumulate (bit set) afterwards.
                    for g in range(4):
                        gg = half * 4 + g
                        ps_ = slice(half * 512 + g * 128, half * 512 + (g + 1) * 128)
                        nc.tensor.matmul(kv_ps[:, gg, :], vsb[:, ps_], khat[:, ps_],
                                         start=(t == 0 and g == 0),
                                         stop=(t == TTILES - 1),
                                         skip_group_check=True)

            # kv -> DRAM bounce -> pairwise AllReduce (diagonal blocks only:
            # pair block rows 0:64 <-> cols 0:64 (head 2g), 64:128 <-> 64:128)
            kv_stage = psmall.tile([128, 8, 64], F32, tag="kvstage")
            nc.vector.tensor_copy(kv_stage[0:64, :, :], kv_ps[0:64, :, 0:64])
            nc.vector.tensor_copy(kv_stage[64:128, :, :], kv_ps[64:128, :, 64:128])
            bounce_in = pdram.tile([128, 8 * 64], F32)
            bounce_out = pdram.tile([128, 8 * 64], F32)
            wq = load_w("wqxT")
            nc.gpsimd.dma_start(out=bounce_in[:], in_=kv_stage[:])
            if dbg:
                nc.sync.dma_start(out=dbg_kvpre[:, :, :], in_=kv_stage[:])
            if use_cc:
                nc.gpsimd.collective_compute(
                    "AllReduce", mybir.AluOpType.add,
                    replica_groups=[[0, 1], [2, 3], [4, 5], [6, 7]],
                    ins=[bounce_in[:].opt()],
                    outs=[bounce_out[:].opt()],
                )
            else:
                bounce_out = bounce_in

            # ---------- phase Q (overlaps the AllReduce) ----------
            qT = pbig.tile([128, DT, TX + TC], BF16, tag="qT")    # 34 KiB/p
            for t in range(TTILES):
                if t == XTILES:
                    wq = load_w("wqcT")
                src, col0 = src_of(t)
                qhat = pact.tile([128, D], BF16, tag="qhat")
                for half in range(2):
                    hs = slice(half * 512, (half + 1) * 512)
                    qp = psA.tile([128, 512], F32, tag="proj")
                    for d in range(DT):
                        nc.tensor.matmul(qp[:], src[:, d, col0:col0 + 128],
                                         wq[:, d, hs], start=(d == 0), stop=(d == DT - 1))
                    l2norm_scaled(qp, qhat, half)
                # transpose q_hat into qT storage
                tcol = t * 128 if t < XTILES else TX
                tps = psT.tile([128, DT, 128], BF16, tag="tps")
                for d in range(DT):
                    nc.tensor.matmul(tps[:, d, :], qhat[:, d * 128:(d + 1) * 128],
                                     ident[:], is_transpose=True)
                nc.scalar.activation(qT[:, :, tcol:tcol + 128], tps[:],
                                     mybir.ActivationFunctionType.Copy)

            # ---------- AllReduce result back + M build ----------
            wox = load_w("woxT")
            woc = load_w("wocT")
            kvr = psmall.tile([128, 8, 64], F32, tag="kvr")
            nc.gpsimd.dma_start(out=kvr[:], in_=bounce_out[:])
            # block-diagonal kv tile: bd[:, g, :] = [[vk_h0, 0], [0, vk_h1]]
            # so one 128-deep matmul per (pair, chunk) builds both heads' M rows
            kvb = psmall.tile([128, 8, 128], BF16, tag="kvb")
            nc.vector.memset(kvb[:], 0.0)
            nc.vector.tensor_copy(kvb[0:64, :, 0:64], kvr[0:64, :, :])
            nc.vector.tensor_copy(kvb[64:128, :, 64:128], kvr[64:128, :, :])
            if dbg:
                nc.sync.dma_start(out=dbg_kv[:, :, 0:64], in_=kvr[:])

            Mx = pbig.tile([128, DT, D], BF16, tag="xsb")   # reuse xsb slot
            Mc = pbig.tile([128, DT, D], BF16, tag="Mc")
            for wo, Mt in ((wox, Mx), (woc, Mc)):
                for g in range(8):
                    mp0 = psA.tile([128, 512], F32, tag="proj")
                    mp1 = psA.tile([128, 512], F32, tag="proj")
                    for mp, chunk in ((mp0, 0), (mp1, 1)):
                        nc.tensor.matmul(
                            mp[:], kvb[:, g, :],
                            wo[:, g, chunk * 512:(chunk + 1) * 512],
                            start=True, stop=True, skip_group_check=True)
                    nc.vector.tensor_copy(Mt[:, g, 0:512], mp0[:])
                    nc.scalar.activation(Mt[:, g, 512:1024], mp1[:],
                                         mybir.ActivationFunctionType.Copy)

            if dbg:
                nc.sync.dma_start(out=dbg_m[:, :, :], in_=Mx[:])
                nc.sync.dma_start(out=dbg_qt[:, :, :], in_=qT[:])

            # ---------- out pass ----------
            # per-half copies alternate DVE/ACT (run concurrently) and each
            # half DMAs out as soon as its copy lands, shortening the tail
            for t in range(TTILES):
                is_c = (t == XTILES)
                tcol = TX if is_c else t * 128
                Mt = Mc if is_c else Mx
                ob = pact.tile([128, D], F32, tag="ob")
                for half in range(2):
                    hs = slice(half * 512, (half + 1) * 512)
                    op = psA.tile([128, 512], F32, tag="proj")
                    for d in range(DT):
                        nc.tensor.matmul(op[:], qT[:, d, tcol:tcol + 128],
                                         Mt[:, d, hs], start=(d == 0), stop=(d == DT - 1))
                    if half == 0:
                        nc.vector.tensor_copy(ob[:, hs], op[:])
                    else:
                        nc.scalar.activation(ob[:, hs], op[:],
                                             mybir.ActivationFunctionType.Copy)
                    dst = outc_d if is_c else outx_d[t * 128:(t + 1) * 128, :]
                    nc.sync.dma_start(out=dst[:, hs], in_=ob[:, hs])

    nc.compile()
    return nc


def _prep_in_maps(x, c, Wq_x, Wk_x, Wv_x, Wq_c, Wk_c, Wv_c, Wo_x, Wo_c,
                  norm_const, TX=None, TC=None):
    import ml_dtypes
    bf = ml_dtypes.bfloat16
    n = x.shape[1]
    m = c.shape[1]
    TX = TX or n * B // NCORES
    TC = TC or m * B // NCORES
    x = np.asarray(x, np.float32)
    c = np.asarray(c, np.float32)
    norm_const = np.asarray(norm_const, np.float32)

    ntot = n + m
    sig = 1.0 / (1.0 + np.exp(-norm_const.astype(np.float64).reshape(H)))
    scale = (float(ntot) ** (-sig)).astype(np.float32)
    svec = np.repeat(scale, HD)            # (1024,) per output dim
    Wv_x_s = np.asarray(Wv_x, np.float32) * svec[:, None]
    Wv_c_s = np.asarray(Wv_c, np.float32) * svec[:, None]

    def wT(W):
        return np.ascontiguousarray(np.asarray(W, np.float32).T.astype(bf))

    weights = {
        "wqxT": wT(Wq_x), "wkxT": wT(Wk_x), "wvxT": wT(Wv_x_s),
        "wqcT": wT(Wq_c), "wkcT": wT(Wk_c), "wvcT": wT(Wv_c_s),
        "woxT": wT(Wo_x), "wocT": wT(Wo_c),
    }

    xr = x.reshape(B, 2, TX, D)
    cr = c.reshape(B, 2, TC, D)
    xtiles = TX // 128
    in_maps = []
    for i in range(NCORES):
        b, hf = i // 2, i % 2
        mm = dict(weights)
        # tile-major: [t, p(dim%128), d(dim//128), j(tok%128)] so each
        # per-tile DMA reads 2 KiB contiguous per partition row
        xs = xr[b, hf].astype(bf)
        mm["xT"] = np.ascontiguousarray(
            xs.reshape(xtiles, 128, DT, 128).transpose(0, 3, 2, 1))
        cs = cr[b, hf].astype(bf)
        mm["cT"] = np.ascontiguousarray(
            cs.reshape(TC, DT, 128).transpose(2, 1, 0))
        in_maps.append(mm)
    return in_maps


def _gather_outputs(results, n=N, m=MC):
    TX = n * B // NCORES
    TC = m * B // NCORES
    out_x = np.empty((B, 2, TX, D), np.float32)
    out_c = np.empty((B, 2, TC, D), np.float32)
    for i in range(NCORES):
        b, hf = i // 2, i % 2
        out_x[b, hf] = results[i]["out_x"]
        out_c[b, hf] = results[i]["out_c"]
    return out_x.reshape(B, n, D), out_c.reshape(B, m, D)


def kernel(x, c, Wq_x, Wk_x, Wv_x, Wq_c, Wk_c, Wv_c, Wo_x, Wo_c, norm_const):
    import os
    from concourse import bass_utils

    if "nc" not in _CACHE:
        _CACHE["nc"] = _build_nc()
    nc = _CACHE["nc"]
    in_maps = _prep_in_maps(x, c, Wq_x, Wk_x, Wv_x, Wq_c, Wk_c, Wv_c,
                            Wo_x, Wo_c, norm_const)
    kwargs = {}
    if os.environ.get("KERNEL_TRACE") == "1":
        try:
            from antenv.axon_hooks import get_axon_ntff_profile_hook
            if get_axon_ntff_profile_hook() is not None:
                kwargs = {"trace": True,
                          "tmpdir": os.environ.get("KERNEL_TRACE_DIR") or None}
        except ImportError:
            pass
    res = bass_utils.run_bass_kernel_spmd(nc, in_maps, core_ids=list(range(NCORES)),
                                          **kwargs)
    _CACHE["last_res"] = res
    return _gather_outputs(res.results)
